# revision 35
# baseline (speedup 1.0000x reference)
"""Bass/Tile Trainium2 kernel for nn_BcosGCNLayer (b-cos linear layer, B=2).

reference:
    lin  = z @ W.T
    cos  = normalize(z) @ normalize(W).T
    out  = lin * |cos|**(B-1) = lin * |cos|          (B = 2)

Key identity used here: with
    W~ = W * ||w_row||^(-1/2)   (row-wise)
    P  = z @ W~.T = lin / sqrt(||w||)        [per column o]
we get  P * |P| * (1/||z_n||) = lin * |lin| / (||z||*||w||) = lin * |cos| = out.
One GEMM; the epilogue is A = |P| * inv_zn (one ACT op — inv_zn is
per-partition in the [n, o] output tile layout, so it rides the
activation's scale operand) followed by out = P * A (one DVE op).

Sharding: data-parallel on rows across 8 cores (12500 rows/core, padded to
12544 = 98*128); weight replicated.

Performance-critical layout: rows are processed in groups of 512 with the
row->partition mapping n = 4p + q (q = 0..3), so one 1MB load/store DMA
moves 8KB CONTIGUOUS per partition (2KB chunks only reach ~171GB/s on the
HBM->SBUF path; 8KB chunks reach ~330GB/s). ACT ops are function-batched
(Square x4, Sqrt, Abs x4 per group) because every activation-function
switch costs ~1us of table reload. GEMMs run in fp32r (full PE rate at
N=512; inputs rounded by the DVE copyback). Loads ride the HWDGE (sync)
queue, stores the SWDGE (gpsimd) queue so a store waiting on compute never
blocks a load.
"""

import numpy as np

import concourse.bacc as bacc
import concourse.bass as bass
import concourse.mybir as mybir
import concourse.tile as tile
from concourse import masks

P = 128
D = 512
KB = D // P  # 4 blocks of 128 along the feature dim
GQ = 4  # rows per partition per group (group = GQ*P = 512 rows)
N_CORES = 8
TOTAL_ROWS = 100000
ROWS_PER_CORE_RAW = TOTAL_ROWS // N_CORES  # 12500
TILES_PER_CORE = -(-ROWS_PER_CORE_RAW // P)  # 98
ROWS_PER_CORE = TILES_PER_CORE * P  # 12544

F32 = mybir.dt.float32
F32R = mybir.dt.float32r
ACT = mybir.ActivationFunctionType

STORE_ENGINE = "gpsimd"
ABS_ON_DVE_EVERY = 0  # every Nth q-slice's abs runs on DVE instead of ACT (0=off)


def build_kernel(
    rows: int = ROWS_PER_CORE,
    repeat: int = 1,
    alias_rows: int = 0,
    hw_loop: int = 0,
) -> bass.Bass:
    """Build the per-core Bass program: z [rows, 512] -> out [rows, 512].

    repeat / alias_rows / hw_loop are bench-only knobs: alias_rows shrinks
    the DRAM tensors (addressing wraps) so host<->device shipping is tiny,
    hw_loop wraps the whole pass in a For_i, repeat emits several passes
    per loop iteration.
    """
    assert rows % P == 0
    n_tiles = rows // P
    dram_rows = alias_rows or rows

    # groups of (tile0, qn): qn*P rows with row mapping n = tile0*P + qn*p + q
    groups = []
    r = 0
    while r < n_tiles:
        qn = min(GQ, n_tiles - r)
        groups.append((r, qn))
        r += qn

    nc = bacc.Bacc()
    z_dram = nc.dram_tensor("z", [dram_rows, D], F32, kind="ExternalInput")
    w_dram = nc.dram_tensor("w", [D, D], F32, kind="ExternalInput")
    out_dram = nc.dram_tensor("out", [dram_rows, D], F32, kind="ExternalOutput")

    def rowslice(dram, t0, qn):
        r0 = (t0 * P) % dram_rows
        return dram[r0 : r0 + qn * P, :].rearrange("(p q) d -> p (q d)", p=P, q=qn)

    with tile.TileContext(nc) as tc:
        with (
            tc.tile_pool(name="consts", bufs=1) as consts,
            tc.tile_pool(name="wprep", bufs=1) as wprep,
            tc.tile_pool(name="zin", bufs=8) as zin_pool,
            tc.tile_pool(name="scratch", bufs=1) as scratch_pool,
            tc.tile_pool(name="stats", bufs=8) as stats_pool,
            tc.tile_pool(name="zt", bufs=22) as zt_pool,
            tc.tile_pool(name="absb", bufs=6) as abs_pool,
            tc.tile_pool(name="outb", bufs=4) as out_pool,
            tc.tile_pool(name="psum_t", bufs=3, space=bass.MemorySpace.PSUM) as pt_pool,
            tc.tile_pool(name="psum_o", bufs=5, space=bass.MemorySpace.PSUM) as po_pool,
        ):
            ident = consts.tile([P, P], F32)
            masks.make_identity(nc, ident[:])
            # PE warmup: absorbs the identity-producer wait into a single
            # instruction so later PE ops carry at most one foreign wait
            # (TPB instructions have exactly one inline sem-wait slot).
            warm = pt_pool.tile([P, P], F32, name="psum_t")
            nc.tensor.transpose(warm[:], ident[:], ident[:])

            # persistent W~T tiles: [i-block k][i=128, o=512]
            wT = wprep.tile([P, KB, D], F32R)

            def batch_front(g, ssq, qoff):
                """One group: 1MB contiguous load, then per q-slice:
                Square-accum (ACT) into ssq[:, qoff+q], 4 PE transposes,
                copyback (q0 on DVE, q1-3 on ACT for engine balance)."""
                t0, qn = g
                zbig = zin_pool.tile([P, GQ, D], F32, name="z_nat")
                nc.sync.dma_start(
                    zbig[:, :qn, :].rearrange("p a b -> p (a b)"),
                    rowslice(z_dram, t0, qn),
                )
                ztiles = []
                for q in range(qn):
                    zq = zbig[:, q, :]
                    zsq_scr = scratch_pool.tile([P, D], F32, name="zsq_scr")
                    nc.scalar.activation(
                        zsq_scr[:], zq, ACT.Square,
                        accum_out=ssq[:, qoff + q : qoff + q + 1],
                    )
                    ptz = pt_pool.tile([P, KB, P], F32, name="psum_t")
                    for k in range(KB):
                        nc.tensor.transpose(
                            ptz[:, k, :], zq[:, k * P : (k + 1) * P], ident[:]
                        )
                    ztile = zt_pool.tile([P, KB, P], F32R, name="ztile")
                    if q != 0:
                        # balance: odd q-slice copybacks ride ACT (Copy needs
                        # no activation table, so no switch penalty)
                        nc.scalar.copy(
                            ztile[:].rearrange("p a b -> p (a b)"),
                            ptz[:].rearrange("p a b -> p (a b)"),
                        )
                    else:
                        nc.vector.tensor_copy(
                            ztile[:].rearrange("p a b -> p (a b)"),
                            ptz[:].rearrange("p a b -> p (a b)"),
                        )
                    ztiles.append(ztile)
                return ztiles

            def stats_chain(ssq, nq):
                """inv_zn = sqrt(1/ssq) for a PAIR of groups: one DVE
                reciprocal + one ACT Sqrt per 8 tiles. Sqrt (ACT) is the
                producer so the Abs scale dep stays same-engine."""
                zrec = stats_pool.tile([P, 2 * GQ], F32, name="zrec")
                nc.vector.reciprocal(zrec[:, :nq], ssq[:, :nq])
                zscale = stats_pool.tile([P, 2 * GQ], F32, name="zscale")
                nc.scalar.activation(zscale[:, :nq], zrec[:, :nq], ACT.Sqrt)
                return zscale

            def batch_back(g, zscale, qoff, ztiles):
                """GEMMs + epilogue + one 1MB store."""
                t0, qn = g
                pos = []
                for q in range(qn):
                    po = po_pool.tile([P, D], F32, name="psum_o")
                    for k in range(KB):
                        nc.tensor.matmul(
                            po[:],
                            ztiles[q][:, k, :],
                            wT[:, k, :],
                            start=(k == 0),
                            stop=(k == KB - 1),
                        )
                    pos.append(po)
                og = out_pool.tile([P, GQ, D], F32, name="ot")
                for q in range(qn):
                    po = pos[q]
                    ab = abs_pool.tile([P, D], F32, name="ab")
                    t = t0 + q
                    if ABS_ON_DVE_EVERY and t % ABS_ON_DVE_EVERY == ABS_ON_DVE_EVERY - 1:
                        nc.vector.tensor_scalar(
                            ab[:], po[:], 0.0, zscale[:, qoff + q : qoff + q + 1],
                            mybir.AluOpType.abs_max, mybir.AluOpType.mult,
                        )
                    else:
                        nc.scalar.activation(
                            ab[:], po[:], ACT.Abs,
                            scale=zscale[:, qoff + q : qoff + q + 1],
                        )
                    nc.vector.tensor_mul(og[:, q, :], po[:], ab[:])
                getattr(nc, STORE_ENGINE).dma_start(
                    rowslice(out_dram, t0, qn),
                    og[:, :qn, :].rearrange("p a b -> p (a b)"),
                )

            def w_prep_stats():
                """W load + norm-scale chain (no PE work): runs while the
                first z groups stream in."""
                w_nat = wprep.tile([P, KB, D], F32)
                nc.sync.dma_start(
                    w_nat[:], w_dram[:].rearrange("(b p) d -> p b d", p=P)
                )
                wsq_scratch = wprep.tile([P, D], F32)
                wssq = wprep.tile([P, KB], F32)
                for b in range(KB):
                    nc.scalar.activation(
                        wsq_scratch[:], w_nat[:, b, :], ACT.Square,
                        accum_out=wssq[:, b : b + 1],
                    )
                wnrm = wprep.tile([P, KB], F32)
                nc.scalar.activation(wnrm[:], wssq[:], ACT.Sqrt)  # ||w||
                wnrm2 = wprep.tile([P, KB], F32)
                nc.scalar.activation(wnrm2[:], wnrm[:], ACT.Sqrt)  # ||w||^(1/2)
                wscale = wprep.tile([P, KB], F32)
                nc.vector.reciprocal(wscale[:], wnrm2[:])  # ||w||^(-1/2)
                # DVE-sourced copies of both W-matmul operands so the W PE
                # matmuls wait on a single engine's semaphore.
                w_nat2 = wprep.tile([P, KB, D], F32)
                nc.vector.tensor_copy(
                    w_nat2[:].rearrange("p a b -> p (a b)"),
                    w_nat[:].rearrange("p a b -> p (a b)"),
                )
                # diag(s_w) per o-block, for the fused scale+transpose matmul
                dsw = wprep.tile([P, KB, P], F32)
                for b in range(KB):
                    nc.vector.tensor_scalar_mul(
                        dsw[:, b, :], ident[:], wscale[:, b : b + 1]
                    )
                return w_nat2, dsw

            def w_prep_pe(w_nat2, dsw):
                """One fused scale+transpose matmul per (o-block, i-block):
                W.T @ diag(s_w) = (s_w * W).T"""
                for k in range(KB):
                    pw = pt_pool.tile([P, KB, P], F32, name="psum_t")
                    for b in range(KB):
                        nc.tensor.matmul(
                            pw[:, b, :],
                            w_nat2[:, b, k * P : (k + 1) * P],
                            dsw[:, b, :],
                        )
                    nc.vector.tensor_copy(
                        wT[:, k, :], pw[:].rearrange("p a b -> p (a b)")
                    )

            LOOKAHEAD = 3

            def emit_passes(n_passes):
                all_groups = groups * n_passes
                n = len(all_groups)
                zts, ssqs = {}, {}

                def do_front(j):
                    pi = j // 2
                    if pi not in ssqs:
                        ssqs[pi] = stats_pool.tile([P, 2 * GQ], F32, name="ssq")
                    zts[j] = batch_front(all_groups[j], ssqs[pi], (j % 2) * GQ)

                for j in range(min(LOOKAHEAD, n)):
                    do_front(j)
                yield  # caller interleaves W-prep PE work here
                i = 0
                while i < n:
                    if i + 1 < n:
                        nq = all_groups[i][1] + all_groups[i + 1][1]
                        zsc = stats_chain(ssqs.pop(i // 2), nq)
                        batch_back(all_groups[i], zsc, 0, zts.pop(i))
                        batch_back(all_groups[i + 1], zsc, GQ, zts.pop(i + 1))
                        step = 2
                    else:
                        zsc = stats_chain(ssqs.pop(i // 2), all_groups[i][1])
                        batch_back(all_groups[i], zsc, 0, zts.pop(i))
                        step = 1
                    for j in range(i + LOOKAHEAD, min(i + LOOKAHEAD + step, n)):
                        if j >= LOOKAHEAD or j not in zts:
                            if j not in zts:
                                do_front(j)
                    i += step

            w_nat2, dsw = w_prep_stats()
            if hw_loop:
                w_prep_pe(w_nat2, dsw)
                with tc.For_i(
                    0, hw_loop, 1,
                    hint_engines=(mybir.EngineType.PE, mybir.EngineType.Activation,
                                  mybir.EngineType.DVE, mybir.EngineType.SP,
                                  mybir.EngineType.Pool),
                ):
                    for _ in emit_passes(repeat):
                        pass
            else:
                gen = emit_passes(repeat)
                next(gen)
                w_prep_pe(w_nat2, dsw)
                for _ in gen:
                    pass

    nc.compile()
    return nc


_NC_CACHE: dict = {}


def _get_nc(rows: int) -> bass.Bass:
    if rows not in _NC_CACHE:
        _NC_CACHE[rows] = build_kernel(rows)
    return _NC_CACHE[rows]


def kernel(z: np.ndarray, weight: np.ndarray) -> np.ndarray:
    """Full-input entry point: z [100000, 512] f32, weight [512, 512] f32."""
    from concourse.bass_utils import run_bass_kernel_spmd

    z = np.ascontiguousarray(z, dtype=np.float32)
    weight = np.ascontiguousarray(weight, dtype=np.float32)
    n_rows = z.shape[0]
    per_core = -(-n_rows // N_CORES)
    per_core_pad = -(-per_core // P) * P

    nc = _get_nc(per_core_pad)

    in_maps = []
    for c in range(N_CORES):
        lo = c * per_core
        hi = min(n_rows, (c + 1) * per_core)
        shard = np.zeros((per_core_pad, D), dtype=np.float32)
        shard[: hi - lo] = z[lo:hi]
        in_maps.append({"z": shard, "w": weight})

    res = run_bass_kernel_spmd(nc, in_maps, core_ids=list(range(N_CORES)))
    out = np.empty((n_rows, D), dtype=np.float32)
    for c in range(N_CORES):
        lo = c * per_core
        hi = min(n_rows, (c + 1) * per_core)
        out[lo:hi] = res.results[c]["out"][: hi - lo]
    return out


# revision 37
# speedup vs baseline: 1.5145x; 1.5145x over previous
"""Bass/Tile Trainium2 kernel for nn_BcosGCNLayer (b-cos linear layer, B=2).

reference:
    lin  = z @ W.T
    cos  = normalize(z) @ normalize(W).T
    out  = lin * |cos|**(B-1) = lin * |cos|          (B = 2)

Key identity used here: with
    W~ = W * ||w_row||^(-1/2)   (row-wise)
    P  = z @ W~.T = lin / sqrt(||w||)        [per column o]
we get  P * |P| * (1/||z_n||) = lin * |lin| / (||z||*||w||) = lin * |cos| = out.
One GEMM; the epilogue is A = |P| * inv_zn (one ACT op — inv_zn is
per-partition in the [n, o] output tile layout, so it rides the
activation's scale operand) followed by out = P * A (one DVE op).

Sharding: data-parallel on rows across 8 cores (12500 rows/core, padded to
12544 = 98*128); weight replicated.

Performance-critical layout: rows are processed in groups of 512 with the
row->partition mapping n = 4p + q (q = 0..3), so one 1MB load/store DMA
moves 8KB CONTIGUOUS per partition (2KB chunks only reach ~171GB/s on the
HBM->SBUF path; 8KB chunks reach ~330GB/s). ACT ops are function-batched
(Square x4, Sqrt, Abs x4 per group) because every activation-function
switch costs ~1us of table reload. GEMMs run in fp32r (full PE rate at
N=512; inputs rounded by the DVE copyback). Loads ride the HWDGE (sync)
queue, stores the SWDGE (gpsimd) queue so a store waiting on compute never
blocks a load.
"""

import numpy as np

import concourse.bacc as bacc
import concourse.bass as bass
import concourse.mybir as mybir
import concourse.tile as tile
from concourse import masks

P = 128
D = 512
KB = D // P  # 4 blocks of 128 along the feature dim
GQ = 4  # rows per partition per group (group = GQ*P = 512 rows)
N_CORES = 8
TOTAL_ROWS = 100000
ROWS_PER_CORE_RAW = TOTAL_ROWS // N_CORES  # 12500
TILES_PER_CORE = -(-ROWS_PER_CORE_RAW // P)  # 98
ROWS_PER_CORE = TILES_PER_CORE * P  # 12544

F32 = mybir.dt.float32
F32R = mybir.dt.float32r
ACT = mybir.ActivationFunctionType

STORE_ENGINE = "gpsimd"
ABS_ON_DVE_EVERY = 0  # every Nth q-slice's abs runs on DVE instead of ACT (0=off)


def build_kernel(
    rows: int = ROWS_PER_CORE,
    repeat: int = 1,
    alias_rows: int = 0,
    hw_loop: int = 0,
) -> bass.Bass:
    """Build the per-core Bass program: z [rows, 512] -> out [rows, 512].

    repeat / alias_rows / hw_loop are bench-only knobs: alias_rows shrinks
    the DRAM tensors (addressing wraps) so host<->device shipping is tiny,
    hw_loop wraps the whole pass in a For_i, repeat emits several passes
    per loop iteration.
    """
    assert rows % P == 0
    n_tiles = rows // P
    dram_rows = alias_rows or rows

    # groups of (tile0, qn): qn*P rows with row mapping n = tile0*P + qn*p + q
    groups = []
    r = 0
    while r < n_tiles:
        qn = min(GQ, n_tiles - r)
        groups.append((r, qn))
        r += qn

    nc = bacc.Bacc()
    z_dram = nc.dram_tensor("z", [dram_rows, D], F32, kind="ExternalInput")
    w_dram = nc.dram_tensor("w", [D, D], F32, kind="ExternalInput")
    out_dram = nc.dram_tensor("out", [dram_rows, D], F32, kind="ExternalOutput")

    def rowslice(dram, t0, qn):
        r0 = (t0 * P) % dram_rows
        return dram[r0 : r0 + qn * P, :].rearrange("(p q) d -> p (q d)", p=P, q=qn)

    with tile.TileContext(nc) as tc:
        with (
            tc.tile_pool(name="consts", bufs=1) as consts,
            tc.tile_pool(name="wprep", bufs=1) as wprep,
            tc.tile_pool(name="zin", bufs=8) as zin_pool,
            tc.tile_pool(name="scratch", bufs=1) as scratch_pool,
            tc.tile_pool(name="stats", bufs=8) as stats_pool,
            tc.tile_pool(name="zt", bufs=14) as zt_pool,
            tc.tile_pool(name="absb", bufs=6) as abs_pool,
            tc.tile_pool(name="outb", bufs=3) as out_pool,
            tc.tile_pool(name="psum_t", bufs=3, space=bass.MemorySpace.PSUM) as pt_pool,
            tc.tile_pool(name="psum_o", bufs=5, space=bass.MemorySpace.PSUM) as po_pool,
        ):
            ident = consts.tile([P, P], F32)
            masks.make_identity(nc, ident[:])
            # PE warmup: absorbs the identity-producer wait into a single
            # instruction so later PE ops carry at most one foreign wait
            # (TPB instructions have exactly one inline sem-wait slot).
            warm = pt_pool.tile([P, P], F32, name="psum_t")
            nc.tensor.transpose(warm[:], ident[:], ident[:])

            # persistent W~T tiles: [i-block k][i=128, o=512]
            wT = wprep.tile([P, KB, D], F32R)

            def batch_front(g):
                """One group: 1MB contiguous load, then per q-slice:
                Square-accum (ACT), 4 PE transposes, DVE copyback."""
                t0, qn = g
                zbig = zin_pool.tile([P, GQ, D], F32, name="z_nat")
                nc.sync.dma_start(
                    zbig[:, :qn, :].rearrange("p a b -> p (a b)"),
                    rowslice(z_dram, t0, qn),
                )
                ssq = stats_pool.tile([P, GQ], F32, name="ssq")
                ztiles = []
                for q in range(qn):
                    zq = zbig[:, q, :]
                    zsq_scr = scratch_pool.tile([P, D], F32, name="zsq_scr")
                    nc.scalar.activation(
                        zsq_scr[:], zq, ACT.Square, accum_out=ssq[:, q : q + 1]
                    )
                    ptz = pt_pool.tile([P, KB, P], F32, name="psum_t")
                    for k in range(KB):
                        nc.tensor.transpose(
                            ptz[:, k, :], zq[:, k * P : (k + 1) * P], ident[:]
                        )
                    ztile = zt_pool.tile([P, KB, P], F32R, name="ztile")
                    if q != 0:
                        # balance: odd q-slice copybacks ride ACT (Copy needs
                        # no activation table, so no switch penalty)
                        nc.scalar.copy(
                            ztile[:].rearrange("p a b -> p (a b)"),
                            ptz[:].rearrange("p a b -> p (a b)"),
                        )
                    else:
                        nc.vector.tensor_copy(
                            ztile[:].rearrange("p a b -> p (a b)"),
                            ptz[:].rearrange("p a b -> p (a b)"),
                        )
                    ztiles.append(ztile)
                return ssq, ztiles

            def batch_back(g, ssq, ztiles):
                """GEMMs + inv-norm + epilogue + one 1MB store."""
                t0, qn = g
                pos = []
                for q in range(qn):
                    po = po_pool.tile([P, D], F32, name="psum_o")
                    for k in range(KB):
                        nc.tensor.matmul(
                            po[:],
                            ztiles[q][:, k, :],
                            wT[:, k, :],
                            start=(k == 0),
                            stop=(k == KB - 1),
                        )
                    pos.append(po)
                # inv_zn = sqrt(1/ssq): DVE reciprocal first so the final
                # ACT op (Sqrt) is the producer -> abs's scale dep stays
                # same-engine and the ACT stream is [Sq xqn][Sqrt][Abs xqn]
                # (every activation-table switch costs ~1us).
                zrec = stats_pool.tile([P, GQ], F32, name="zrec")
                nc.vector.reciprocal(zrec[:, :qn], ssq[:, :qn])
                zscale = stats_pool.tile([P, GQ], F32, name="zscale")
                nc.scalar.activation(zscale[:, :qn], zrec[:, :qn], ACT.Sqrt)
                og = out_pool.tile([P, GQ, D], F32, name="ot")
                for q in range(qn):
                    po = pos[q]
                    ab = abs_pool.tile([P, D], F32, name="ab")
                    t = t0 + q
                    if ABS_ON_DVE_EVERY and t % ABS_ON_DVE_EVERY == ABS_ON_DVE_EVERY - 1:
                        nc.vector.tensor_scalar(
                            ab[:], po[:], 0.0, zscale[:, q : q + 1],
                            mybir.AluOpType.abs_max, mybir.AluOpType.mult,
                        )
                    else:
                        nc.scalar.activation(
                            ab[:], po[:], ACT.Abs, scale=zscale[:, q : q + 1]
                        )
                    nc.vector.tensor_mul(og[:, q, :], po[:], ab[:])
                getattr(nc, STORE_ENGINE).dma_start(
                    rowslice(out_dram, t0, qn),
                    og[:, :qn, :].rearrange("p a b -> p (a b)"),
                )

            def w_prep_stats():
                """W load + norm-scale chain (no PE work): runs while the
                first z groups stream in."""
                w_nat = wprep.tile([P, KB, D], F32)
                nc.sync.dma_start(
                    w_nat[:], w_dram[:].rearrange("(b p) d -> p b d", p=P)
                )
                wsq_scratch = wprep.tile([P, D], F32)
                wssq = wprep.tile([P, KB], F32)
                for b in range(KB):
                    nc.scalar.activation(
                        wsq_scratch[:], w_nat[:, b, :], ACT.Square,
                        accum_out=wssq[:, b : b + 1],
                    )
                wnrm = wprep.tile([P, KB], F32)
                nc.scalar.activation(wnrm[:], wssq[:], ACT.Sqrt)  # ||w||
                wnrm2 = wprep.tile([P, KB], F32)
                nc.scalar.activation(wnrm2[:], wnrm[:], ACT.Sqrt)  # ||w||^(1/2)
                wscale = wprep.tile([P, KB], F32)
                nc.vector.reciprocal(wscale[:], wnrm2[:])  # ||w||^(-1/2)
                # DVE-sourced copies of both W-matmul operands so the W PE
                # matmuls wait on a single engine's semaphore.
                w_nat2 = wprep.tile([P, KB, D], F32)
                nc.vector.tensor_copy(
                    w_nat2[:].rearrange("p a b -> p (a b)"),
                    w_nat[:].rearrange("p a b -> p (a b)"),
                )
                # diag(s_w) per o-block, for the fused scale+transpose matmul
                dsw = wprep.tile([P, KB, P], F32)
                for b in range(KB):
                    nc.vector.tensor_scalar_mul(
                        dsw[:, b, :], ident[:], wscale[:, b : b + 1]
                    )
                return w_nat2, dsw

            def w_prep_pe(w_nat2, dsw):
                """One fused scale+transpose matmul per (o-block, i-block):
                W.T @ diag(s_w) = (s_w * W).T"""
                for k in range(KB):
                    pw = pt_pool.tile([P, KB, P], F32, name="psum_t")
                    for b in range(KB):
                        nc.tensor.matmul(
                            pw[:, b, :],
                            w_nat2[:, b, k * P : (k + 1) * P],
                            dsw[:, b, :],
                        )
                    nc.vector.tensor_copy(
                        wT[:, k, :], pw[:].rearrange("p a b -> p (a b)")
                    )

            LOOKAHEAD = 3

            def emit_passes(n_passes):
                all_groups = groups * n_passes
                fronts = {}
                for i in range(min(LOOKAHEAD, len(all_groups))):
                    fronts[i] = batch_front(all_groups[i])
                yield  # caller interleaves W-prep PE work here
                for i in range(len(all_groups)):
                    ssq, ztiles = fronts.pop(i)
                    batch_back(all_groups[i], ssq, ztiles)
                    if i + LOOKAHEAD < len(all_groups):
                        fronts[i + LOOKAHEAD] = batch_front(all_groups[i + LOOKAHEAD])

            w_nat2, dsw = w_prep_stats()
            if hw_loop:
                w_prep_pe(w_nat2, dsw)
                with tc.For_i(
                    0, hw_loop, 1,
                    hint_engines=(mybir.EngineType.PE, mybir.EngineType.Activation,
                                  mybir.EngineType.DVE, mybir.EngineType.SP,
                                  mybir.EngineType.Pool),
                ):
                    for _ in emit_passes(repeat):
                        pass
            else:
                gen = emit_passes(repeat)
                next(gen)
                w_prep_pe(w_nat2, dsw)
                for _ in gen:
                    pass

    nc.compile()
    return nc


_NC_CACHE: dict = {}


def _get_nc(rows: int) -> bass.Bass:
    if rows not in _NC_CACHE:
        _NC_CACHE[rows] = build_kernel(rows)
    return _NC_CACHE[rows]


def kernel(z: np.ndarray, weight: np.ndarray) -> np.ndarray:
    """Full-input entry point: z [100000, 512] f32, weight [512, 512] f32."""
    from concourse.bass_utils import run_bass_kernel_spmd

    z = np.ascontiguousarray(z, dtype=np.float32)
    weight = np.ascontiguousarray(weight, dtype=np.float32)
    n_rows = z.shape[0]
    per_core = -(-n_rows // N_CORES)
    per_core_pad = -(-per_core // P) * P

    nc = _get_nc(per_core_pad)

    in_maps = []
    for c in range(N_CORES):
        lo = c * per_core
        hi = min(n_rows, (c + 1) * per_core)
        shard = np.zeros((per_core_pad, D), dtype=np.float32)
        shard[: hi - lo] = z[lo:hi]
        in_maps.append({"z": shard, "w": weight})

    res = run_bass_kernel_spmd(nc, in_maps, core_ids=list(range(N_CORES)))
    out = np.empty((n_rows, D), dtype=np.float32)
    for c in range(N_CORES):
        lo = c * per_core
        hi = min(n_rows, (c + 1) * per_core)
        out[lo:hi] = res.results[c]["out"][: hi - lo]
    return out


# revision 39
# speedup vs baseline: 1.6124x; 1.0646x over previous
"""Bass/Tile Trainium2 kernel for nn_BcosGCNLayer (b-cos linear layer, B=2).

reference:
    lin  = z @ W.T
    cos  = normalize(z) @ normalize(W).T
    out  = lin * |cos|**(B-1) = lin * |cos|          (B = 2)

Key identity used here: with
    W~ = W * ||w_row||^(-1/2)   (row-wise)
    P  = z @ W~.T = lin / sqrt(||w||)        [per column o]
we get  P * |P| * (1/||z_n||) = lin * |lin| / (||z||*||w||) = lin * |cos| = out.
One GEMM; the epilogue is A = |P| * inv_zn (one ACT op — inv_zn is
per-partition in the [n, o] output tile layout, so it rides the
activation's scale operand) followed by out = P * A (one DVE op).

Sharding: data-parallel on rows across 8 cores (12500 rows/core, padded to
12544 = 98*128); weight replicated.

Performance-critical layout: rows are processed in groups of 512 with the
row->partition mapping n = 4p + q (q = 0..3), so one 1MB load/store DMA
moves 8KB CONTIGUOUS per partition (2KB chunks only reach ~171GB/s on the
HBM->SBUF path; 8KB chunks reach ~330GB/s). ACT ops are function-batched
(Square x4, Sqrt, Abs x4 per group) because every activation-function
switch costs ~1us of table reload. GEMMs run in fp32r (full PE rate at
N=512; inputs rounded by the DVE copyback). Loads ride the HWDGE (sync)
queue, stores the SWDGE (gpsimd) queue so a store waiting on compute never
blocks a load.
"""

import numpy as np

import concourse.bacc as bacc
import concourse.bass as bass
import concourse.mybir as mybir
import concourse.tile as tile
from concourse import masks

P = 128
D = 512
KB = D // P  # 4 blocks of 128 along the feature dim
GQ = 4  # rows per partition per group (group = GQ*P = 512 rows)
N_CORES = 8
TOTAL_ROWS = 100000
ROWS_PER_CORE_RAW = TOTAL_ROWS // N_CORES  # 12500
TILES_PER_CORE = -(-ROWS_PER_CORE_RAW // P)  # 98
ROWS_PER_CORE = TILES_PER_CORE * P  # 12544

F32 = mybir.dt.float32
F32R = mybir.dt.float32r
ACT = mybir.ActivationFunctionType

STORE_ENGINE = "gpsimd"
ABS_ON_DVE_EVERY = 0  # every Nth q-slice's abs runs on DVE instead of ACT (0=off)


def build_kernel(
    rows: int = ROWS_PER_CORE,
    repeat: int = 1,
    alias_rows: int = 0,
    hw_loop: int = 0,
) -> bass.Bass:
    """Build the per-core Bass program: z [rows, 512] -> out [rows, 512].

    repeat / alias_rows / hw_loop are bench-only knobs: alias_rows shrinks
    the DRAM tensors (addressing wraps) so host<->device shipping is tiny,
    hw_loop wraps the whole pass in a For_i, repeat emits several passes
    per loop iteration.
    """
    assert rows % P == 0
    n_tiles = rows // P
    dram_rows = alias_rows or rows

    # groups of (tile0, qn): qn*P rows with row mapping n = tile0*P + qn*p + q
    groups = []
    r = 0
    while r < n_tiles:
        qn = min(GQ, n_tiles - r)
        groups.append((r, qn))
        r += qn

    nc = bacc.Bacc()
    z_dram = nc.dram_tensor("z", [dram_rows, D], F32, kind="ExternalInput")
    w_dram = nc.dram_tensor("w", [D, D], F32, kind="ExternalInput")
    out_dram = nc.dram_tensor("out", [dram_rows, D], F32, kind="ExternalOutput")

    def rowslice(dram, t0, qn):
        r0 = (t0 * P) % dram_rows
        return dram[r0 : r0 + qn * P, :].rearrange("(p q) d -> p (q d)", p=P, q=qn)

    with tile.TileContext(nc) as tc:
        with (
            tc.tile_pool(name="consts", bufs=1) as consts,
            tc.tile_pool(name="wprep", bufs=1) as wprep,
            tc.tile_pool(name="zin", bufs=10) as zin_pool,
            tc.tile_pool(name="scratch", bufs=1) as scratch_pool,
            tc.tile_pool(name="stats", bufs=8) as stats_pool,
            tc.tile_pool(name="zt", bufs=14) as zt_pool,
            tc.tile_pool(name="absb", bufs=6) as abs_pool,
            tc.tile_pool(name="outb", bufs=4) as out_pool,
            tc.tile_pool(name="psum_t", bufs=3, space=bass.MemorySpace.PSUM) as pt_pool,
            tc.tile_pool(name="psum_o", bufs=5, space=bass.MemorySpace.PSUM) as po_pool,
        ):
            ident = consts.tile([P, P], F32)
            masks.make_identity(nc, ident[:])
            # PE warmup: absorbs the identity-producer wait into a single
            # instruction so later PE ops carry at most one foreign wait
            # (TPB instructions have exactly one inline sem-wait slot).
            warm = pt_pool.tile([P, P], F32, name="psum_t")
            nc.tensor.transpose(warm[:], ident[:], ident[:])

            # persistent W~T tiles: [i-block k][i=128, o=512]
            wT = wprep.tile([P, KB, D], F32R)

            def batch_front(g):
                """One group: 1MB contiguous load, then per q-slice:
                Square-accum (ACT), 4 PE transposes, DVE copyback."""
                t0, qn = g
                zbig = zin_pool.tile([P, GQ, D], F32, name="z_nat")
                nc.sync.dma_start(
                    zbig[:, :qn, :].rearrange("p a b -> p (a b)"),
                    rowslice(z_dram, t0, qn),
                )
                ssq = stats_pool.tile([P, GQ], F32, name="ssq")
                ztiles = []
                for q in range(qn):
                    zq = zbig[:, q, :]
                    zsq_scr = scratch_pool.tile([P, D], F32, name="zsq_scr")
                    nc.scalar.activation(
                        zsq_scr[:], zq, ACT.Square, accum_out=ssq[:, q : q + 1]
                    )
                    ptz = pt_pool.tile([P, KB, P], F32, name="psum_t")
                    for k in range(KB):
                        nc.tensor.transpose(
                            ptz[:, k, :], zq[:, k * P : (k + 1) * P], ident[:]
                        )
                    ztile = zt_pool.tile([P, KB, P], F32R, name="ztile")
                    if q % 2:
                        # balance: odd q-slice copybacks ride ACT (Copy needs
                        # no activation table, so no switch penalty)
                        nc.scalar.copy(
                            ztile[:].rearrange("p a b -> p (a b)"),
                            ptz[:].rearrange("p a b -> p (a b)"),
                        )
                    else:
                        nc.vector.tensor_copy(
                            ztile[:].rearrange("p a b -> p (a b)"),
                            ptz[:].rearrange("p a b -> p (a b)"),
                        )
                    ztiles.append(ztile)
                return ssq, ztiles

            def stats_chain(ssq, qn):
                """inv_zn = sqrt(1/ssq). DVE reciprocal first so the final
                ACT op (Sqrt) is the producer -> abs's scale dep stays
                same-engine. Called for PAIRS of groups back-to-back so the
                two Sqrts are adjacent in the ACT stream (one switch)."""
                zrec = stats_pool.tile([P, GQ], F32, name="zrec")
                nc.vector.reciprocal(zrec[:, :qn], ssq[:, :qn])
                zscale = stats_pool.tile([P, GQ], F32, name="zscale")
                nc.scalar.activation(zscale[:, :qn], zrec[:, :qn], ACT.Sqrt)
                return zscale

            def batch_back(g, zscale, ztiles):
                """GEMMs + epilogue + one 1MB store."""
                t0, qn = g
                pos = []
                for q in range(qn):
                    po = po_pool.tile([P, D], F32, name="psum_o")
                    for k in range(KB):
                        nc.tensor.matmul(
                            po[:],
                            ztiles[q][:, k, :],
                            wT[:, k, :],
                            start=(k == 0),
                            stop=(k == KB - 1),
                        )
                    pos.append(po)
                og = out_pool.tile([P, GQ, D], F32, name="ot")
                for q in range(qn):
                    po = pos[q]
                    ab = abs_pool.tile([P, D], F32, name="ab")
                    t = t0 + q
                    if ABS_ON_DVE_EVERY and t % ABS_ON_DVE_EVERY == ABS_ON_DVE_EVERY - 1:
                        nc.vector.tensor_scalar(
                            ab[:], po[:], 0.0, zscale[:, q : q + 1],
                            mybir.AluOpType.abs_max, mybir.AluOpType.mult,
                        )
                    else:
                        nc.scalar.activation(
                            ab[:], po[:], ACT.Abs, scale=zscale[:, q : q + 1]
                        )
                    nc.vector.tensor_mul(og[:, q, :], po[:], ab[:])
                getattr(nc, STORE_ENGINE).dma_start(
                    rowslice(out_dram, t0, qn),
                    og[:, :qn, :].rearrange("p a b -> p (a b)"),
                )

            def w_prep_stats():
                """W load + norm-scale chain (no PE work): runs while the
                first z groups stream in."""
                w_nat = wprep.tile([P, KB, D], F32)
                nc.sync.dma_start(
                    w_nat[:], w_dram[:].rearrange("(b p) d -> p b d", p=P)
                )
                wsq_scratch = wprep.tile([P, D], F32)
                wssq = wprep.tile([P, KB], F32)
                for b in range(KB):
                    nc.scalar.activation(
                        wsq_scratch[:], w_nat[:, b, :], ACT.Square,
                        accum_out=wssq[:, b : b + 1],
                    )
                wnrm = wprep.tile([P, KB], F32)
                nc.scalar.activation(wnrm[:], wssq[:], ACT.Sqrt)  # ||w||
                wnrm2 = wprep.tile([P, KB], F32)
                nc.scalar.activation(wnrm2[:], wnrm[:], ACT.Sqrt)  # ||w||^(1/2)
                wscale = wprep.tile([P, KB], F32)
                nc.vector.reciprocal(wscale[:], wnrm2[:])  # ||w||^(-1/2)
                # DVE-sourced copies of both W-matmul operands so the W PE
                # matmuls wait on a single engine's semaphore.
                w_nat2 = wprep.tile([P, KB, D], F32)
                nc.vector.tensor_copy(
                    w_nat2[:].rearrange("p a b -> p (a b)"),
                    w_nat[:].rearrange("p a b -> p (a b)"),
                )
                # diag(s_w) per o-block, for the fused scale+transpose matmul
                dsw = wprep.tile([P, KB, P], F32)
                for b in range(KB):
                    nc.vector.tensor_scalar_mul(
                        dsw[:, b, :], ident[:], wscale[:, b : b + 1]
                    )
                return w_nat2, dsw

            def w_prep_pe(w_nat2, dsw):
                """One fused scale+transpose matmul per (o-block, i-block):
                W.T @ diag(s_w) = (s_w * W).T"""
                for k in range(KB):
                    pw = pt_pool.tile([P, KB, P], F32, name="psum_t")
                    for b in range(KB):
                        nc.tensor.matmul(
                            pw[:, b, :],
                            w_nat2[:, b, k * P : (k + 1) * P],
                            dsw[:, b, :],
                        )
                    nc.vector.tensor_copy(
                        wT[:, k, :], pw[:].rearrange("p a b -> p (a b)")
                    )

            LOOKAHEAD = 3

            def emit_passes(n_passes):
                all_groups = groups * n_passes
                fronts = {}
                for i in range(min(LOOKAHEAD, len(all_groups))):
                    fronts[i] = batch_front(all_groups[i])
                yield  # caller interleaves W-prep PE work here
                zscales = {}
                for i in range(len(all_groups)):
                    ssq, ztiles = fronts.pop(i)
                    if i not in zscales:
                        zscales[i] = stats_chain(ssq, all_groups[i][1])
                        if i + 1 in fronts:
                            zscales[i + 1] = stats_chain(
                                fronts[i + 1][0], all_groups[i + 1][1]
                            )
                    batch_back(all_groups[i], zscales.pop(i), ztiles)
                    if i + LOOKAHEAD < len(all_groups):
                        fronts[i + LOOKAHEAD] = batch_front(all_groups[i + LOOKAHEAD])

            w_nat2, dsw = w_prep_stats()
            if hw_loop:
                w_prep_pe(w_nat2, dsw)
                with tc.For_i(
                    0, hw_loop, 1,
                    hint_engines=(mybir.EngineType.PE, mybir.EngineType.Activation,
                                  mybir.EngineType.DVE, mybir.EngineType.SP,
                                  mybir.EngineType.Pool),
                ):
                    for _ in emit_passes(repeat):
                        pass
            else:
                gen = emit_passes(repeat)
                next(gen)
                w_prep_pe(w_nat2, dsw)
                for _ in gen:
                    pass

    nc.compile()
    return nc


_NC_CACHE: dict = {}


def _get_nc(rows: int) -> bass.Bass:
    if rows not in _NC_CACHE:
        _NC_CACHE[rows] = build_kernel(rows)
    return _NC_CACHE[rows]


def kernel(z: np.ndarray, weight: np.ndarray) -> np.ndarray:
    """Full-input entry point: z [100000, 512] f32, weight [512, 512] f32."""
    from concourse.bass_utils import run_bass_kernel_spmd

    z = np.ascontiguousarray(z, dtype=np.float32)
    weight = np.ascontiguousarray(weight, dtype=np.float32)
    n_rows = z.shape[0]
    per_core = -(-n_rows // N_CORES)
    per_core_pad = -(-per_core // P) * P

    nc = _get_nc(per_core_pad)

    in_maps = []
    for c in range(N_CORES):
        lo = c * per_core
        hi = min(n_rows, (c + 1) * per_core)
        shard = np.zeros((per_core_pad, D), dtype=np.float32)
        shard[: hi - lo] = z[lo:hi]
        in_maps.append({"z": shard, "w": weight})

    res = run_bass_kernel_spmd(nc, in_maps, core_ids=list(range(N_CORES)))
    out = np.empty((n_rows, D), dtype=np.float32)
    for c in range(N_CORES):
        lo = c * per_core
        hi = min(n_rows, (c + 1) * per_core)
        out[lo:hi] = res.results[c]["out"][: hi - lo]
    return out


# revision 40
# speedup vs baseline: 1.6273x; 1.0092x over previous
"""Bass/Tile Trainium2 kernel for nn_BcosGCNLayer (b-cos linear layer, B=2).

reference:
    lin  = z @ W.T
    cos  = normalize(z) @ normalize(W).T
    out  = lin * |cos|**(B-1) = lin * |cos|          (B = 2)

Key identity used here: with
    W~ = W * ||w_row||^(-1/2)   (row-wise)
    P  = z @ W~.T = lin / sqrt(||w||)        [per column o]
we get  P * |P| * (1/||z_n||) = lin * |lin| / (||z||*||w||) = lin * |cos| = out.
One GEMM; the epilogue is A = |P| * inv_zn (one ACT op — inv_zn is
per-partition in the [n, o] output tile layout, so it rides the
activation's scale operand) followed by out = P * A (one DVE op).

Sharding: data-parallel on rows across 8 cores (12500 rows/core, padded to
12544 = 98*128); weight replicated.

Performance-critical layout: rows are processed in groups of 512 with the
row->partition mapping n = 4p + q (q = 0..3), so one 1MB load/store DMA
moves 8KB CONTIGUOUS per partition (2KB chunks only reach ~171GB/s on the
HBM->SBUF path; 8KB chunks reach ~330GB/s). ACT ops are function-batched
(Square x4, Sqrt, Abs x4 per group) because every activation-function
switch costs ~1us of table reload. GEMMs run in fp32r (full PE rate at
N=512; inputs rounded by the DVE copyback). Loads ride the HWDGE (sync)
queue, stores the SWDGE (gpsimd) queue so a store waiting on compute never
blocks a load.
"""

import numpy as np

import concourse.bacc as bacc
import concourse.bass as bass
import concourse.mybir as mybir
import concourse.tile as tile
from concourse import masks

P = 128
D = 512
KB = D // P  # 4 blocks of 128 along the feature dim
GQ = 4  # rows per partition per group (group = GQ*P = 512 rows)
N_CORES = 8
TOTAL_ROWS = 100000
ROWS_PER_CORE_RAW = TOTAL_ROWS // N_CORES  # 12500
TILES_PER_CORE = -(-ROWS_PER_CORE_RAW // P)  # 98
ROWS_PER_CORE = TILES_PER_CORE * P  # 12544

F32 = mybir.dt.float32
F32R = mybir.dt.float32r
ACT = mybir.ActivationFunctionType

STORE_ENGINE = "gpsimd"
ABS_ON_DVE_EVERY = 0  # every Nth q-slice's abs runs on DVE instead of ACT (0=off)


def build_kernel(
    rows: int = ROWS_PER_CORE,
    repeat: int = 1,
    alias_rows: int = 0,
    hw_loop: int = 0,
) -> bass.Bass:
    """Build the per-core Bass program: z [rows, 512] -> out [rows, 512].

    repeat / alias_rows / hw_loop are bench-only knobs: alias_rows shrinks
    the DRAM tensors (addressing wraps) so host<->device shipping is tiny,
    hw_loop wraps the whole pass in a For_i, repeat emits several passes
    per loop iteration.
    """
    assert rows % P == 0
    n_tiles = rows // P
    dram_rows = alias_rows or rows

    # groups of (tile0, qn): qn*P rows with row mapping n = tile0*P + qn*p + q
    groups = []
    r = 0
    while r < n_tiles:
        qn = min(GQ, n_tiles - r)
        groups.append((r, qn))
        r += qn

    nc = bacc.Bacc()
    z_dram = nc.dram_tensor("z", [dram_rows, D], F32, kind="ExternalInput")
    w_dram = nc.dram_tensor("w", [D, D], F32, kind="ExternalInput")
    out_dram = nc.dram_tensor("out", [dram_rows, D], F32, kind="ExternalOutput")

    def rowslice(dram, t0, qn):
        r0 = (t0 * P) % dram_rows
        return dram[r0 : r0 + qn * P, :].rearrange("(p q) d -> p (q d)", p=P, q=qn)

    with tile.TileContext(nc) as tc:
        with (
            tc.tile_pool(name="consts", bufs=1) as consts,
            tc.tile_pool(name="wprep", bufs=1) as wprep,
            tc.tile_pool(name="zin", bufs=8) as zin_pool,
            tc.tile_pool(name="scratch", bufs=1) as scratch_pool,
            tc.tile_pool(name="stats", bufs=8) as stats_pool,
            tc.tile_pool(name="zt", bufs=14) as zt_pool,
            tc.tile_pool(name="absb", bufs=6) as abs_pool,
            tc.tile_pool(name="outb", bufs=3) as out_pool,
            tc.tile_pool(name="psum_t", bufs=3, space=bass.MemorySpace.PSUM) as pt_pool,
            tc.tile_pool(name="psum_o", bufs=5, space=bass.MemorySpace.PSUM) as po_pool,
        ):
            ident = consts.tile([P, P], F32)
            masks.make_identity(nc, ident[:])
            # PE warmup: absorbs the identity-producer wait into a single
            # instruction so later PE ops carry at most one foreign wait
            # (TPB instructions have exactly one inline sem-wait slot).
            warm = pt_pool.tile([P, P], F32, name="psum_t")
            nc.tensor.transpose(warm[:], ident[:], ident[:])

            # persistent W~T tiles: [i-block k][i=128, o=512]
            wT = wprep.tile([P, KB, D], F32R)

            def batch_front(g):
                """One group: 1MB contiguous load, then per q-slice:
                Square-accum (ACT), 4 PE transposes, DVE copyback."""
                t0, qn = g
                zbig = zin_pool.tile([P, GQ, D], F32, name="z_nat")
                nc.sync.dma_start(
                    zbig[:, :qn, :].rearrange("p a b -> p (a b)"),
                    rowslice(z_dram, t0, qn),
                )
                ssq = stats_pool.tile([P, GQ], F32, name="ssq")
                ztiles = []
                for q in range(qn):
                    zq = zbig[:, q, :]
                    zsq_scr = scratch_pool.tile([P, D], F32, name="zsq_scr")
                    nc.scalar.activation(
                        zsq_scr[:], zq, ACT.Square, accum_out=ssq[:, q : q + 1]
                    )
                    ptz = pt_pool.tile([P, KB, P], F32, name="psum_t")
                    for k in range(KB):
                        nc.tensor.transpose(
                            ptz[:, k, :], zq[:, k * P : (k + 1) * P], ident[:]
                        )
                    ztile = zt_pool.tile([P, KB, P], F32R, name="ztile")
                    if q % 2:
                        # balance: odd q-slice copybacks ride ACT (Copy needs
                        # no activation table, so no switch penalty)
                        nc.scalar.copy(
                            ztile[:].rearrange("p a b -> p (a b)"),
                            ptz[:].rearrange("p a b -> p (a b)"),
                        )
                    else:
                        nc.vector.tensor_copy(
                            ztile[:].rearrange("p a b -> p (a b)"),
                            ptz[:].rearrange("p a b -> p (a b)"),
                        )
                    ztiles.append(ztile)
                return ssq, ztiles

            def batch_back(g, ssq, ztiles):
                """GEMMs + inv-norm + epilogue + one 1MB store."""
                t0, qn = g
                pos = []
                for q in range(qn):
                    po = po_pool.tile([P, D], F32, name="psum_o")
                    for k in range(KB):
                        nc.tensor.matmul(
                            po[:],
                            ztiles[q][:, k, :],
                            wT[:, k, :],
                            start=(k == 0),
                            stop=(k == KB - 1),
                        )
                    pos.append(po)
                # inv_zn = sqrt(1/ssq): DVE reciprocal first so the final
                # ACT op (Sqrt) is the producer -> abs's scale dep stays
                # same-engine and the ACT stream is [Sq xqn][Sqrt][Abs xqn]
                # (every activation-table switch costs ~1us).
                zrec = stats_pool.tile([P, GQ], F32, name="zrec")
                nc.vector.reciprocal(zrec[:, :qn], ssq[:, :qn])
                zscale = stats_pool.tile([P, GQ], F32, name="zscale")
                nc.scalar.activation(zscale[:, :qn], zrec[:, :qn], ACT.Sqrt)
                og = out_pool.tile([P, GQ, D], F32, name="ot")
                for q in range(qn):
                    po = pos[q]
                    ab = abs_pool.tile([P, D], F32, name="ab")
                    t = t0 + q
                    if ABS_ON_DVE_EVERY and t % ABS_ON_DVE_EVERY == ABS_ON_DVE_EVERY - 1:
                        nc.vector.tensor_scalar(
                            ab[:], po[:], 0.0, zscale[:, q : q + 1],
                            mybir.AluOpType.abs_max, mybir.AluOpType.mult,
                        )
                    else:
                        nc.scalar.activation(
                            ab[:], po[:], ACT.Abs, scale=zscale[:, q : q + 1]
                        )
                    nc.vector.tensor_mul(og[:, q, :], po[:], ab[:])
                getattr(nc, STORE_ENGINE).dma_start(
                    rowslice(out_dram, t0, qn),
                    og[:, :qn, :].rearrange("p a b -> p (a b)"),
                )

            def w_prep_stats():
                """W load + norm-scale chain (no PE work): runs while the
                first z groups stream in."""
                w_nat = wprep.tile([P, KB, D], F32)
                nc.sync.dma_start(
                    w_nat[:], w_dram[:].rearrange("(b p) d -> p b d", p=P)
                )
                wsq_scratch = wprep.tile([P, D], F32)
                wssq = wprep.tile([P, KB], F32)
                for b in range(KB):
                    nc.scalar.activation(
                        wsq_scratch[:], w_nat[:, b, :], ACT.Square,
                        accum_out=wssq[:, b : b + 1],
                    )
                wnrm = wprep.tile([P, KB], F32)
                nc.scalar.activation(wnrm[:], wssq[:], ACT.Sqrt)  # ||w||
                wnrm2 = wprep.tile([P, KB], F32)
                nc.scalar.activation(wnrm2[:], wnrm[:], ACT.Sqrt)  # ||w||^(1/2)
                wscale = wprep.tile([P, KB], F32)
                nc.vector.reciprocal(wscale[:], wnrm2[:])  # ||w||^(-1/2)
                # DVE-sourced copies of both W-matmul operands so the W PE
                # matmuls wait on a single engine's semaphore.
                w_nat2 = wprep.tile([P, KB, D], F32)
                nc.vector.tensor_copy(
                    w_nat2[:].rearrange("p a b -> p (a b)"),
                    w_nat[:].rearrange("p a b -> p (a b)"),
                )
                # diag(s_w) per o-block, for the fused scale+transpose matmul
                dsw = wprep.tile([P, KB, P], F32)
                for b in range(KB):
                    nc.vector.tensor_scalar_mul(
                        dsw[:, b, :], ident[:], wscale[:, b : b + 1]
                    )
                return w_nat2, dsw

            def w_prep_pe(w_nat2, dsw):
                """One fused scale+transpose matmul per (o-block, i-block):
                W.T @ diag(s_w) = (s_w * W).T"""
                for k in range(KB):
                    pw = pt_pool.tile([P, KB, P], F32, name="psum_t")
                    for b in range(KB):
                        nc.tensor.matmul(
                            pw[:, b, :],
                            w_nat2[:, b, k * P : (k + 1) * P],
                            dsw[:, b, :],
                        )
                    nc.vector.tensor_copy(
                        wT[:, k, :], pw[:].rearrange("p a b -> p (a b)")
                    )

            LOOKAHEAD = 3

            def emit_passes(n_passes):
                all_groups = groups * n_passes
                fronts = {}
                for i in range(min(LOOKAHEAD, len(all_groups))):
                    fronts[i] = batch_front(all_groups[i])
                yield  # caller interleaves W-prep PE work here
                for i in range(len(all_groups)):
                    ssq, ztiles = fronts.pop(i)
                    batch_back(all_groups[i], ssq, ztiles)
                    if i + LOOKAHEAD < len(all_groups):
                        fronts[i + LOOKAHEAD] = batch_front(all_groups[i + LOOKAHEAD])

            w_nat2, dsw = w_prep_stats()
            if hw_loop:
                w_prep_pe(w_nat2, dsw)
                with tc.For_i(
                    0, hw_loop, 1,
                    hint_engines=(mybir.EngineType.PE, mybir.EngineType.Activation,
                                  mybir.EngineType.DVE, mybir.EngineType.SP,
                                  mybir.EngineType.Pool),
                ):
                    for _ in emit_passes(repeat):
                        pass
            else:
                gen = emit_passes(repeat)
                next(gen)
                w_prep_pe(w_nat2, dsw)
                for _ in gen:
                    pass

    nc.compile()
    return nc


_NC_CACHE: dict = {}


def _get_nc(rows: int) -> bass.Bass:
    if rows not in _NC_CACHE:
        _NC_CACHE[rows] = build_kernel(rows)
    return _NC_CACHE[rows]


def kernel(z: np.ndarray, weight: np.ndarray) -> np.ndarray:
    """Full-input entry point: z [100000, 512] f32, weight [512, 512] f32."""
    from concourse.bass_utils import run_bass_kernel_spmd

    z = np.ascontiguousarray(z, dtype=np.float32)
    weight = np.ascontiguousarray(weight, dtype=np.float32)
    n_rows = z.shape[0]
    per_core = -(-n_rows // N_CORES)
    per_core_pad = -(-per_core // P) * P

    nc = _get_nc(per_core_pad)

    in_maps = []
    for c in range(N_CORES):
        lo = c * per_core
        hi = min(n_rows, (c + 1) * per_core)
        shard = np.zeros((per_core_pad, D), dtype=np.float32)
        shard[: hi - lo] = z[lo:hi]
        in_maps.append({"z": shard, "w": weight})

    res = run_bass_kernel_spmd(nc, in_maps, core_ids=list(range(N_CORES)))
    out = np.empty((n_rows, D), dtype=np.float32)
    for c in range(N_CORES):
        lo = c * per_core
        hi = min(n_rows, (c + 1) * per_core)
        out[lo:hi] = res.results[c]["out"][: hi - lo]
    return out
